# revision 8
# baseline (speedup 1.0000x reference)
"""DCNv2 (modulated deformable conv) + BN + SiLU Trainium2 Bass kernel.

V2: byte-pair packed gathers + fused unpack-multiply lerp.

x is quantized host-side to s8 (x*32, clip +-127). Each token row of
x_tok packs, per channel, TWO horizontally adjacent pixels in one i16:
  low byte  = u8  biased (+128) value of (y, x)
  high byte = s8  value of (y, x+1)
So ONE 512B gather descriptor fetches BOTH left+right corners for all
256 channels -> gather DMA bytes halve vs bf16 corners.

The bilinear lerp runs as scalar_tensor_tensor ops on DVE (4x mode,
0.26 ns/elem): (tok & 255) * w00  and  (tok >> 8) * w01 fuse the byte
unpack into the corner-weight product. Weights (incl sigmoid mask and
the /32 dequant scale) are fp16, broadcast per k as in V1.

The +128 low-byte bias is linear: out_err = sum_k 4*(sigma*olx)_k[pix]
* S_k[o], S_k[o] = sum_c W[o,c,k]. It is removed INSIDE the einsum
PSUM accumulation with 8 extra matmuls: lhsT = -4*S (host-side), rhs =
mT[9, 2048] = sigma*olx transposed on PE.

Sharding: 8 cores = batch (4) x row-half (2); core computes
out[b, :, 32r:32r+32, :]. Everything else (offset conv, index wrap,
BN+SiLU tail) follows V1.
"""

import os
import numpy as np
import ml_dtypes

B, C1, C2, H, W = 4, 256, 256, 64, 64
MAX_OFF = 6.0
BN_EPS = 1e-5

NCORES = 8
HL = 32
P = HL * W              # 2048 pixels / core
GR, GC = 48, 128        # rows h0-8 .. h0+39 (all sampled rows)
PAD = 8

BF16 = ml_dtypes.bfloat16
FP16 = np.float16
QSCALE = 32.0


def _build_nc():
    import concourse.bacc as bacc
    import concourse.mybir as mybir
    import concourse.tile as tile

    f32 = mybir.dt.float32
    bf16 = mybir.dt.bfloat16
    fp16 = mybir.dt.float16
    i16 = mybir.dt.int16

    nc = bacc.Bacc("TRN2", target_bir_lowering=False, debug=False)

    x_tok_d = nc.dram_tensor("x_tok", [128, GR, 256], i16, kind="ExternalInput")
    x_conv_d = nc.dram_tensor("x_conv", [2, 128, 34, 66], bf16, kind="ExternalInput")
    w_om_d = nc.dram_tensor("w_om", [9, 2, 128, 27], bf16, kind="ExternalInput")
    w_dcn_d = nc.dram_tensor("w_dcn", [9, 2, 2, 128, 128], fp16, kind="ExternalInput")
    s4_d = nc.dram_tensor("s4", [2, 9, 128], fp16, kind="ExternalInput")
    base_y_d = nc.dram_tensor("base_y", [128, 9, 16], f32, kind="ExternalInput")
    base_x_d = nc.dram_tensor("base_x", [128, 9, 16], f32, kind="ExternalInput")
    bias_y_d = nc.dram_tensor("bias_y", [128, 9, 16], f32, kind="ExternalInput")
    bias_x_d = nc.dram_tensor("bias_x", [128, 9, 16], f32, kind="ExternalInput")
    bias_m_d = nc.dram_tensor("bias_m", [128, 9, 16], f32, kind="ExternalInput")
    ident_d = nc.dram_tensor("ident", [128, 128], bf16, kind="ExternalInput")
    identh_d = nc.dram_tensor("identh", [128, 128], fp16, kind="ExternalInput")
    cst_d = nc.dram_tensor("cst", [128, 2], i16, kind="ExternalInput")
    bn_d = nc.dram_tensor("bn", [4, 128, 2], f32, kind="ExternalInput")
    out_d = nc.dram_tensor("out", [2, 128, P], f32, kind="ExternalOutput")
    w_stage_d = nc.dram_tensor("w_stage", [9, 4 * P], fp16)

    with tile.TileContext(nc) as tc:
        with tc.tile_pool(name="persist", bufs=1) as big:
            x_tok = big.tile([128, GR, 256], i16)
            nc.sync.dma_start(x_tok[:], x_tok_d[:])
            wd = big.tile([128, 9, 2, 2, 128], fp16)
            nc.sync.dma_start(wd[:], w_dcn_d[:].rearrange("k c o p q -> p k c o q"))
            ident = big.tile([128, 128], bf16)
            nc.scalar.dma_start(ident[:], ident_d[:])
            identh = big.tile([128, 128], fp16)
            nc.scalar.dma_start(identh[:], identh_d[:])
            cst = big.tile([128, 2], i16)
            nc.scalar.dma_start(cst[:], cst_d[:])
            s4sb = big.tile([9, 2, 128], fp16)
            nc.scalar.dma_start(s4sb[:], s4_d[:].rearrange("a p b -> p a b"))
            wrap_rep = big.tile([128, 9, 256], i16)
            mT = big.tile([9, 16, 128], fp16)
            bn_s = big.tile([128, 2], f32)
            bn_o = big.tile([128, 2], f32)
            _phase1(nc, tc, mybir, big, x_tok, wd, ident, identh, wrap_rep, mT, bn_s, bn_o,
                    x_conv_d, w_om_d, base_y_d, base_x_d, bias_y_d, bias_x_d,
                    bias_m_d, bn_d, w_stage_d)
            _phase2(nc, tc, mybir, x_tok, wd, ident, wrap_rep, mT, s4sb, cst,
                    bn_s, bn_o, big, w_stage_d, out_d)

    nc.compile()
    return nc


def _phase1(nc, tc, mybir, big, x_tok, wd, ident, identh, wrap_rep, mT, bn_s, bn_o,
            x_conv_d, w_om_d, base_y_d, base_x_d, bias_y_d, bias_x_d,
            bias_m_d, bn_d, w_stage_d):
    f32 = mybir.dt.float32
    bf16 = mybir.dt.bfloat16
    fp16 = mybir.dt.float16
    i16 = mybir.dt.int16
    AF = mybir.ActivationFunctionType
    OP = mybir.AluOpType
    with (
        tc.tile_pool(name="chain", bufs=1) as chain,
        tc.tile_pool(name="psum", bufs=1, space="PSUM") as psp,
    ):
        # ---------- static loads (conv inputs first: critical path) ----------
        xc = chain.tile([128, 2, 34, 66], bf16)
        nc.scalar.dma_start(xc[:], x_conv_d[:].rearrange("c p a b -> p c a b"))
        w_om = chain.tile([128, 9, 2, 27], bf16)
        nc.scalar.dma_start(w_om[:], w_om_d[:].rearrange("k c p o -> p k c o"))
        base_y = chain.tile([128, 9, 16], f32, tag="base_y")
        nc.sync.dma_start(base_y[:], base_y_d[:])
        base_x = chain.tile([128, 9, 16], f32, tag="base_x")
        nc.sync.dma_start(base_x[:], base_x_d[:])
        bias_y = chain.tile([128, 9, 16], f32, tag="bias_y")
        nc.sync.dma_start(bias_y[:], bias_y_d[:])
        bias_x = chain.tile([128, 9, 16], f32, tag="bias_x")
        nc.sync.dma_start(bias_x[:], bias_x_d[:])
        bias_m = chain.tile([128, 9, 16], f32, tag="bias_m")
        nc.sync.dma_start(bias_m[:], bias_m_d[:])
        bn_in = chain.tile([128, 4, 2], f32, tag="bn_in")
        nc.sync.dma_start(bn_in[:], bn_d[:].rearrange("a p b -> p a b"))

        # ---------- BN constants on device ----------
        tvar = chain.tile([128, 2], f32, tag="tvar")
        nc.vector.tensor_scalar(tvar[:], bn_in[:, 3], BN_EPS, None, OP.add)
        nc.scalar.sqrt(tvar[:], tvar[:])
        nc.vector.reciprocal(tvar[:], tvar[:])
        nc.vector.tensor_tensor(bn_s[:], bn_in[:, 0], tvar[:], OP.mult)
        nc.vector.tensor_tensor(bn_o[:], bn_in[:, 2], bn_s[:], OP.mult)
        nc.vector.tensor_tensor(bn_o[:], bn_in[:, 1], bn_o[:], OP.subtract)

        # ---------- 1. offset conv ----------
        om_ps = psp.tile([27, P], f32, tag="pa")
        for ky in range(3):
            for kx in range(3):
                k = ky * 3 + kx
                for ct in range(2):
                    for n in range(4):
                        nc.tensor.matmul(
                            om_ps[:, n * 512:(n + 1) * 512],
                            w_om[:, k, ct],
                            xc[:, ct, ky + n * 8: ky + n * 8 + 8, kx: kx + 64],
                            start=(k == 0 and ct == 0),
                            stop=(k == 8 and ct == 1),
                        )
        om_sb = chain.tile([27, P], bf16, tag="om_sb")
        nc.scalar.copy(om_sb[:], om_ps[:])

        # ---------- 2. PE transpose om -> [128, 16, 27] ----------
        omT_ps = psp.tile([128, 16 * 28], bf16, tag="pb")
        for ch in range(16):
            nc.tensor.transpose(
                omT_ps[:, ch * 28:ch * 28 + 27],
                om_sb[:, ch * 128:(ch + 1) * 128],
                ident[:27, :27],
            )
        omT = chain.tile([128, 16, 27], f32, tag="omT")
        nc.scalar.copy(
            omT[:],
            omT_ps[:].rearrange("p (a b) -> p a b", a=16)[:, :, 0:27],
        )

        # ---------- 3. elementwise chain [128, 9, 16] ----------
        def ct_(name):
            return chain.tile([128, 9, 16], f32, tag=name, name=name)

        dy = ct_("dy"); dx = ct_("dx"); mm = ct_("mm")
        omT_r = omT[:].rearrange("p c o -> p o c")
        nc.vector.tensor_copy(dy[:], omT_r[:, 0:18:2, :])
        nc.vector.tensor_copy(dx[:], omT_r[:, 1:18:2, :])
        nc.vector.tensor_copy(mm[:], omT_r[:, 18:27, :])

        t0 = ct_("t0"); t1 = ct_("t1")
        nc.vector.tensor_tensor(dy[:], dy[:], bias_y[:], OP.add)
        nc.vector.tensor_tensor(dx[:], dx[:], bias_x[:], OP.add)
        nc.vector.tensor_tensor(mm[:], mm[:], bias_m[:], OP.add)
        nc.vector.tensor_scalar(t0[:], dy[:], MAX_OFF, -MAX_OFF, OP.min, OP.max)
        nc.vector.tensor_scalar(t1[:], dx[:], MAX_OFF, -MAX_OFF, OP.min, OP.max)
        pys = ct_("pys"); pxs = ct_("pxs")
        nc.vector.tensor_tensor(pys[:], t0[:], base_y[:], OP.add)
        nc.vector.tensor_tensor(pxs[:], t1[:], base_x[:], OP.add)
        ly = ct_("ly"); lx = ct_("lx")
        y0 = ct_("y0"); x0 = ct_("x0")
        iy = chain.tile([128, 9, 16], mybir.dt.int32, tag="iy", name="iy")
        ix = chain.tile([128, 9, 16], mybir.dt.int32, tag="ix", name="ix")
        # floor(pys) robust to converter rounding mode
        nc.vector.tensor_copy(iy[:], pys[:])
        nc.vector.tensor_copy(y0[:], iy[:])
        nc.vector.tensor_tensor(t0[:], y0[:], pys[:], OP.is_gt)
        nc.vector.tensor_tensor(y0[:], y0[:], t0[:], OP.subtract)
        nc.vector.tensor_tensor(ly[:], pys[:], y0[:], OP.subtract)
        nc.vector.tensor_copy(ix[:], pxs[:])
        nc.vector.tensor_copy(x0[:], ix[:])
        nc.vector.tensor_tensor(t1[:], x0[:], pxs[:], OP.is_gt)
        nc.vector.tensor_tensor(x0[:], x0[:], t1[:], OP.subtract)
        nc.vector.tensor_tensor(lx[:], pxs[:], x0[:], OP.subtract)
        # indices first: the gather critical path starts here
        idxf = ct_("idxf")
        nc.vector.tensor_scalar(t0[:], y0[:], 128.0, None, OP.mult)
        nc.vector.tensor_tensor(idxf[:], t0[:], x0[:], OP.add)
        idx_all = chain.tile([128, 2, 9, 16], i16, tag="idx_all")
        for cr, off in enumerate([0.0, 128.0]):
            nc.vector.tensor_scalar(t1[:], idxf[:], off, None, OP.add)
            nc.vector.tensor_copy(idx_all[:, cr], t1[:])
        msk = ct_("msk")
        nc.scalar.activation(msk[:], mm[:], AF.Sigmoid)
        olx = ct_("olx"); oly32 = ct_("oly32"); ly32 = ct_("ly32")
        nc.vector.tensor_scalar(olx[:], lx[:], -1.0, 1.0, OP.mult, OP.add)
        nc.vector.tensor_scalar(oly32[:], ly[:], -1.0 / QSCALE, 1.0 / QSCALE,
                                OP.mult, OP.add)
        nc.vector.tensor_scalar(ly32[:], ly[:], 1.0 / QSCALE, None, OP.mult)
        wyt = ct_("wyt"); wyb = ct_("wyb")
        nc.vector.tensor_tensor(wyt[:], oly32[:], msk[:], OP.mult)
        nc.vector.tensor_tensor(wyb[:], ly32[:], msk[:], OP.mult)
        wf = chain.tile([128, 4, 9, 16], fp16, tag="wf")
        nc.vector.tensor_tensor(wf[:, 0], wyt[:], olx[:], OP.mult)
        nc.vector.tensor_tensor(wf[:, 1], wyt[:], lx[:], OP.mult)
        nc.vector.tensor_tensor(wf[:, 2], wyb[:], olx[:], OP.mult)
        nc.vector.tensor_tensor(wf[:, 3], wyb[:], lx[:], OP.mult)
        # bias-correction tile: sigma * (1-lx), fp16
        mcorr = chain.tile([128, 9, 16], fp16, tag="mcorr")
        nc.vector.tensor_tensor(mcorr[:], msk[:], olx[:], OP.mult)

        # ---------- 4. wrap16 indices (2 streams: top pair, bottom pair) ----
        # wrap16[q, k, cr*128 + ph*16 + fl] = idx_all[ph*16+q, cr, k, fl]
        wrap16 = chain.tile([16, 9, 2, 8, 16], i16, tag="wrap16")
        for ph in range(8):
            for cr in range(2):
                eng = nc.sync if (ph * 2 + cr) % 2 == 0 else nc.scalar
                eng.dma_start(
                    wrap16[:, :, cr, ph, :],
                    idx_all[ph * 16:(ph + 1) * 16, cr],
                )
        for g8 in range(8):
            eng = nc.sync if g8 % 2 == 0 else nc.scalar
            eng.dma_start(
                wrap_rep[g8 * 16:(g8 + 1) * 16],
                wrap16[:].rearrange("q k cr ph fl -> q k (cr ph fl)"),
            )

        # ---------- 5. weight transpose + staging; mT transpose ----------
        for k in range(9):
            wT_ps = psp.tile([16, 4 * 128], fp16, tag="pc", name=f"wT_ps{k}")
            for cr in range(4):
                nc.tensor.transpose(
                    wT_ps[:, cr * 128:(cr + 1) * 128],
                    wf[:, cr, k, :],
                    identh[:],
                )
            wT = chain.tile([16, 4, 8, 16], fp16, tag="wT", name=f"wT{k}")
            nc.scalar.copy(wT[:].rearrange("p a b c -> p (a b c)"), wT_ps[:])
            eng = nc.sync if k % 2 == 0 else nc.scalar
            eng.dma_start(
                w_stage_d[k].rearrange("(cr ph fl pl) -> fl cr ph pl", cr=4, ph=8, fl=16),
                wT[:],
            )
        # mT[j, f, p] = mcorr[p, j, f]  (pix = f*128 + p)
        mT_ps = psp.tile([9, 16 * 128], fp16, tag="pd")
        for f in range(16):
            nc.tensor.transpose(
                mT_ps[:, f * 128:(f + 1) * 128],
                mcorr[:, :, f],
                identh[:],
            )
        nc.scalar.copy(mT[:].rearrange("p a b -> p (a b)"), mT_ps[:])


def _phase2(nc, tc, mybir, x_tok, wd, ident, wrap_rep, mT, s4sb, cst,
            bn_s, bn_o, big2_outer, w_stage_d, out_d):
    f32 = mybir.dt.float32
    fp16 = mybir.dt.float16
    i16 = mybir.dt.int16
    AF = mybir.ActivationFunctionType
    OP = mybir.AluOpType
    with (
        tc.tile_pool(name="big2", bufs=1) as big2,
        tc.tile_pool(name="gbuf", bufs=2) as gbuf,
        tc.tile_pool(name="wrepp", bufs=2) as wrepp,
        tc.tile_pool(name="colp", bufs=2) as colp,
        tc.tile_pool(name="tmp", bufs=2) as tmpp,
        tc.tile_pool(name="psum2", bufs=1, space="PSUM") as psp2,
    ):
        out_ps = [psp2.tile([128, P], f32, tag=f"o{ot}", name=f"out_ps{ot}")
                  for ot in range(2)]
        # bias-correction matmuls open the PSUM accumulation groups
        # chain pixel (p=ph*16+q, fl) sits at einsum free pos ph*256+fl*16+q
        mT_r = mT[:].rearrange("p fl (ph q) -> p ph fl q", ph=8)
        for ot in range(2):
            for n in range(4):
                nc.tensor.matmul(
                    out_ps[ot][:, n * 512:(n + 1) * 512],
                    s4sb[:, ot],
                    mT_r[:, 2 * n:2 * n + 2],
                    start=True,
                    stop=False,
                )
        for k in range(9):
            w_rep = wrepp.tile([128, 4 * P], fp16, tag="w_rep", name=f"w_rep{k}")
            nc.sync.dma_start(
                w_rep[:],
                w_stage_d[k].partition_broadcast(128),
            )
            ghs = []
            for half in range(2):
                gh = gbuf.tile([128, 2, P], i16, tag=f"g{half}", name=f"g{k}_{half}")
                nc.gpsimd.dma_gather(
                    gh[:],
                    x_tok[:].rearrange("p r c -> p (r c)"),
                    wrap_rep[:, k, half * 128:(half + 1) * 128],
                    P,
                    P,
                    256,
                    transpose=True,
                    sbuf_tokens_per_rank=128,
                    sbuf_free_dim_per_rank=512,
                    single_packet=False,
                )
                ghs.append(gh)
            col = colp.tile([128, 2, P], fp16, tag="col", name=f"col{k}")
            hi_t = tmpp.tile([128, 2, P], i16, tag="hi_t", name=f"hi_t{k}")
            hi_b = tmpp.tile([128, 2, P], i16, tag="hi_b", name=f"hi_b{k}")
            ta = tmpp.tile([128, P], fp16, tag="ta", name=f"ta{k}")
            tb = tmpp.tile([128, P], fp16, tag="tb", name=f"tb{k}")
            c255 = cst[:, 0:1]
            c8 = cst[:, 1:2]
            # unpack: hi = tok >> 8 (signed), then lo = tok & 255 in-place
            nc.vector.tensor_scalar(hi_t[:], ghs[0][:], c8, None,
                                    OP.bitwise_and)
            nc.vector.tensor_scalar(ghs[0][:], ghs[0][:], c255, None,
                                    OP.bitwise_and)
            nc.vector.tensor_scalar(hi_b[:], ghs[1][:], c8, None,
                                    OP.bitwise_and)
            nc.vector.tensor_scalar(ghs[1][:], ghs[1][:], c255, None,
                                    OP.bitwise_and)
            w00 = w_rep[:, 0:P]
            w01 = w_rep[:, P:2 * P]
            w10 = w_rep[:, 2 * P:3 * P]
            w11 = w_rep[:, 3 * P:4 * P]
            for ctile in range(2):
                cc = col[:, ctile]
                nc.vector.scalar_tensor_tensor(ta[:], ghs[0][:, ctile], 1.0,
                                               w00, OP.mult, OP.mult)
                nc.vector.scalar_tensor_tensor(tb[:], hi_t[:, ctile], 1.0 / 256.0,
                                               w01, OP.mult, OP.mult)
                nc.vector.scalar_tensor_tensor(cc, ta[:], 1.0, tb[:],
                                               OP.mult, OP.add)
                nc.vector.scalar_tensor_tensor(ta[:], ghs[1][:, ctile], 1.0,
                                               w10, OP.mult, OP.mult)
                nc.vector.scalar_tensor_tensor(tb[:], hi_b[:, ctile], 1.0 / 256.0,
                                               w11, OP.mult, OP.mult)
                nc.vector.scalar_tensor_tensor(ta[:], ta[:], 1.0, tb[:],
                                               OP.mult, OP.add)
                nc.vector.scalar_tensor_tensor(cc, cc, 1.0, ta[:],
                                               OP.mult, OP.add)
            _emit_einsum(nc, col, wd, out_ps, k)

        # ---------- 7. BN + SiLU + unpermute + store ----------
        for ot in range(2):
            yv = big2.tile([128, P], f32, tag="yv", name=f"yv{ot}")
            sg = big2.tile([128, P], f32, tag="sg", name=f"sg{ot}")
            o_sb = big2.tile([128, P], f32, tag=f"osb{ot}", name=f"o_sb{ot}")
            nc.vector.tensor_scalar(
                yv[:], out_ps[ot][:],
                bn_s[:, ot:ot + 1], bn_o[:, ot:ot + 1],
                OP.mult, OP.add,
            )
            nc.scalar.activation(sg[:], yv[:], AF.Sigmoid)
            nc.vector.tensor_tensor(
                o_sb[:].rearrange("p (c b a) -> p c b a", c=16, b=8),
                yv[:].rearrange("p (b c a) -> p c b a", b=8, c=16),
                sg[:].rearrange("p (b c a) -> p c b a", b=8, c=16),
                OP.mult,
            )
            nc.sync.dma_start(out_d[ot], o_sb[:])


def _emit_einsum(nc, col, wd, out_ps, k):
    for ctile in range(2):
        for ot in range(2):
            for n in range(4):
                nc.tensor.matmul(
                    out_ps[ot][:, n * 512:(n + 1) * 512],
                    wd[:, k, ctile, ot],
                    col[:, ctile, n * 512:(n + 1) * 512],
                    start=False,
                    stop=(k == 8 and ctile == 1),
                )


def _prep_core_inputs(inputs, b, r):
    x = np.asarray(inputs["x"])
    w_om = np.asarray(inputs["w_om"])
    b_om = np.asarray(inputs["b_om"])
    w_dcn = np.asarray(inputs["w_dcn"])
    h0 = HL * r

    # ---- packed token grid: low byte = u8(+128) of (y,x), high = s8 of (y,x+1)
    xq_lo = np.full((GR, GC, 256), 128, dtype=np.uint16)
    xq_hi = np.zeros((GR, GC, 256), dtype=np.uint16)
    y_lo, y_hi = max(0, h0 - PAD), min(H, h0 + HL + PAD)
    xs = x[b][:, y_lo:y_hi, :].transpose(1, 2, 0)          # [rows, 64, 256]
    q = np.clip(np.rint(xs * QSCALE), -127, 127).astype(np.int16)
    r0 = y_lo - (h0 - PAD)
    r1 = y_hi - (h0 - PAD)
    xq_lo[r0:r1, PAD:PAD + W, :] = (q + 128).astype(np.uint16)
    xq_hi[r0:r1, PAD - 1:PAD + W - 1, :] = (q.astype(np.uint8).astype(np.uint16) << 8)
    xq = (xq_lo | xq_hi).view(np.int16)
    x_tok = np.ascontiguousarray(xq.swapaxes(0, 1))        # [128, 48, 256] i16

    xcv = np.zeros((256, 34, 66), dtype=BF16)
    r_lo, r_hi = max(0, h0 - 1), min(H, h0 + 33)
    xcv[:, r_lo - (h0 - 1):r_hi - (h0 - 1), 1:65] = x[b][:, r_lo:r_hi, :].astype(BF16)
    x_conv = np.ascontiguousarray(xcv.reshape(2, 128, 34, 66))

    wl = np.zeros((9, 2, 128, 27), dtype=BF16)
    for ky in range(3):
        for kx in range(3):
            k = ky * 3 + kx
            for ctile in range(2):
                wl[k, ctile] = w_om[:, ctile * 128:(ctile + 1) * 128, ky, kx].T.astype(BF16)

    wdl = np.zeros((9, 2, 2, 128, 128), dtype=FP16)
    wr = w_dcn.reshape(C2, C1, 9)
    for k in range(9):
        for ctile in range(2):
            for ot in range(2):
                wdl[k, ctile, ot] = wr[ot * 128:(ot + 1) * 128,
                                       ctile * 128:(ctile + 1) * 128, k].T.astype(FP16)

    # correction lhsT: s4[ot, k, o] = -4 * sum_c W[o, c, k]
    S = w_dcn.reshape(C2, C1, 9).sum(axis=1)               # [C2, 9]
    s4 = np.zeros((2, 9, 128), dtype=FP16)
    for ot in range(2):
        s4[ot] = (-4.0 * S[ot * 128:(ot + 1) * 128, :].T).astype(FP16)

    p_ = np.arange(128)[:, None, None]
    k_ = np.arange(9)[None, :, None]
    fl = np.arange(16)[None, None, :]
    pix = fl * 128 + p_
    h_loc = pix // W
    w_pix = pix % W
    ky_ = k_ // 3
    kx_ = k_ % 3
    base_y = np.broadcast_to(h_loc + ky_ - 1 + PAD, (128, 9, 16)).astype(np.float32)
    base_x = np.broadcast_to(w_pix + kx_ - 1 + PAD, (128, 9, 16)).astype(np.float32)
    bias_y = np.broadcast_to(b_om[0:18:2][None, :, None], (128, 9, 16)).astype(np.float32)
    bias_x = np.broadcast_to(b_om[1:18:2][None, :, None], (128, 9, 16)).astype(np.float32)
    bias_m = np.broadcast_to(b_om[18:27][None, :, None], (128, 9, 16)).astype(np.float32)

    bn = np.stack([
        np.asarray(inputs["bn_gamma"]).reshape(2, 128).T,
        np.asarray(inputs["bn_beta"]).reshape(2, 128).T,
        np.asarray(inputs["bn_mean"]).reshape(2, 128).T,
        np.asarray(inputs["bn_var"]).reshape(2, 128).T,
    ], axis=0).astype(np.float32)

    cstv = np.tile(np.array([[255, -256]], np.int16), (128, 1))

    return {
        "x_tok": x_tok,
        "x_conv": x_conv,
        "w_om": wl,
        "w_dcn": wdl,
        "s4": s4,
        "base_y": np.ascontiguousarray(base_y),
        "base_x": np.ascontiguousarray(base_x),
        "bias_y": np.ascontiguousarray(bias_y),
        "bias_x": np.ascontiguousarray(bias_x),
        "bias_m": np.ascontiguousarray(bias_m),
        "ident": np.eye(128, dtype=BF16),
        "identh": np.eye(128, dtype=FP16),
        "cst": cstv,
        "bn": np.ascontiguousarray(bn),
    }


_NC_CACHE = {}


def _get_nc():
    if "nc" not in _NC_CACHE:
        _NC_CACHE["nc"] = _build_nc()
    return _NC_CACHE["nc"]


def _assemble(results):
    out = np.zeros((B, C2, H, W), dtype=np.float32)
    for c in range(NCORES):
        b, r = c // 2, c % 2
        o = np.asarray(results[c]["out"])
        for ot in range(2):
            out[b, ot * 128:(ot + 1) * 128, HL * r:HL * (r + 1), :] = (
                o[ot].reshape(128, HL, W).astype(np.float32)
            )
    return out


def _run(inputs, trace=False):
    from concourse.bass_utils import run_bass_kernel_spmd
    nc = _get_nc()
    in_maps = [_prep_core_inputs(inputs, c // 2, c % 2) for c in range(NCORES)]
    res = run_bass_kernel_spmd(nc, in_maps, list(range(NCORES)), trace=trace)
    return _assemble(res.results), res


def kernel(**inputs):
    out, _ = _run(inputs, trace=False)
    return out


# revision 11
# speedup vs baseline: 1.4040x; 1.4040x over previous
"""DCNv2 (modulated deformable conv) + BN + SiLU Trainium2 Bass kernel.

V2: byte-pair packed gathers + fused unpack-multiply lerp.

x is quantized host-side to s8 (x*32, clip +-127). Each token row of
x_tok packs, per channel, TWO horizontally adjacent pixels in one i16:
  low byte  = u8  biased (+128) value of (y, x)
  high byte = s8  value of (y, x+1)
So ONE 512B gather descriptor fetches BOTH left+right corners for all
256 channels -> gather DMA bytes halve vs bf16 corners.

The bilinear lerp runs as scalar_tensor_tensor ops on DVE (4x mode,
0.26 ns/elem): (tok & 255) * w00  and  (tok >> 8) * w01 fuse the byte
unpack into the corner-weight product. Weights (incl sigmoid mask and
the /32 dequant scale) are fp16, broadcast per k as in V1.

The +128 low-byte bias is linear: out_err = sum_k 4*(sigma*olx)_k[pix]
* S_k[o], S_k[o] = sum_c W[o,c,k]. It is removed INSIDE the einsum
PSUM accumulation with 8 extra matmuls: lhsT = -4*S (host-side), rhs =
mT[9, 2048] = sigma*olx transposed on PE.

Sharding: 8 cores = batch (4) x row-half (2); core computes
out[b, :, 32r:32r+32, :]. Everything else (offset conv, index wrap,
BN+SiLU tail) follows V1.
"""

import os
import numpy as np
import ml_dtypes

B, C1, C2, H, W = 4, 256, 256, 64, 64
MAX_OFF = 6.0
BN_EPS = 1e-5

NCORES = 8
HL = 32
P = HL * W              # 2048 pixels / core
GR, GC = 48, 128        # rows h0-8 .. h0+39 (all sampled rows)
PAD = 8

BF16 = ml_dtypes.bfloat16
FP16 = np.float16
QSCALE = 32.0


def _build_nc():
    import concourse.bacc as bacc
    import concourse.mybir as mybir
    import concourse.tile as tile

    f32 = mybir.dt.float32
    bf16 = mybir.dt.bfloat16
    fp16 = mybir.dt.float16
    i16 = mybir.dt.int16

    nc = bacc.Bacc("TRN2", target_bir_lowering=False, debug=False)

    x_tok_d = nc.dram_tensor("x_tok", [128, GR, 256], i16, kind="ExternalInput")
    x_conv_d = nc.dram_tensor("x_conv", [2, 128, 34, 66], bf16, kind="ExternalInput")
    w_om_d = nc.dram_tensor("w_om", [9, 2, 128, 27], bf16, kind="ExternalInput")
    w_dcn_d = nc.dram_tensor("w_dcn", [9, 2, 2, 128, 128], fp16, kind="ExternalInput")
    s4_d = nc.dram_tensor("s4", [2, 9, 128], fp16, kind="ExternalInput")
    base_y_d = nc.dram_tensor("base_y", [128, 9, 16], f32, kind="ExternalInput")
    base_x_d = nc.dram_tensor("base_x", [128, 9, 16], f32, kind="ExternalInput")
    bias_y_d = nc.dram_tensor("bias_y", [128, 9, 16], f32, kind="ExternalInput")
    bias_x_d = nc.dram_tensor("bias_x", [128, 9, 16], f32, kind="ExternalInput")
    bias_m_d = nc.dram_tensor("bias_m", [128, 9, 16], f32, kind="ExternalInput")
    ident_d = nc.dram_tensor("ident", [128, 128], bf16, kind="ExternalInput")
    identh_d = nc.dram_tensor("identh", [128, 128], fp16, kind="ExternalInput")
    cst_d = nc.dram_tensor("cst", [128, 2], i16, kind="ExternalInput")
    bn_d = nc.dram_tensor("bn", [4, 128, 2], f32, kind="ExternalInput")
    out_d = nc.dram_tensor("out", [2, 128, P], f32, kind="ExternalOutput")
    w_stage_d = nc.dram_tensor("w_stage", [9, 2 * P], fp16)
    w_stageh_d = nc.dram_tensor("w_stageh", [9, 2 * P], bf16)

    with tile.TileContext(nc) as tc:
        with tc.tile_pool(name="persist", bufs=1) as big:
            x_tok = big.tile([128, GR, 256], i16)
            nc.sync.dma_start(x_tok[:], x_tok_d[:])
            wd = big.tile([128, 9, 2, 2, 128], fp16)
            nc.sync.dma_start(wd[:], w_dcn_d[:].rearrange("k c o p q -> p k c o q"))
            ident = big.tile([128, 128], bf16)
            nc.scalar.dma_start(ident[:], ident_d[:])
            identh = big.tile([128, 128], fp16)
            nc.scalar.dma_start(identh[:], identh_d[:])
            cst = big.tile([128, 2], i16)
            nc.scalar.dma_start(cst[:], cst_d[:])
            s4sb = big.tile([9, 2, 128], fp16)
            nc.scalar.dma_start(s4sb[:], s4_d[:].rearrange("a p b -> p a b"))
            wrap_rep = big.tile([128, 9, 256], i16)
            mT = big.tile([9, 16, 128], fp16)
            bn_s = big.tile([128, 2], f32)
            bn_o = big.tile([128, 2], f32)
            _phase1(nc, tc, mybir, big, x_tok, wd, ident, identh, wrap_rep, mT, bn_s, bn_o,
                    x_conv_d, w_om_d, base_y_d, base_x_d, bias_y_d, bias_x_d,
                    bias_m_d, bn_d, w_stage_d, w_stageh_d)
            _phase2(nc, tc, mybir, x_tok, wd, ident, wrap_rep, mT, s4sb, cst,
                    bn_s, bn_o, big, w_stage_d, w_stageh_d, out_d)

    nc.compile()
    return nc


def _phase1(nc, tc, mybir, big, x_tok, wd, ident, identh, wrap_rep, mT, bn_s, bn_o,
            x_conv_d, w_om_d, base_y_d, base_x_d, bias_y_d, bias_x_d,
            bias_m_d, bn_d, w_stage_d, w_stageh_d):
    f32 = mybir.dt.float32
    bf16 = mybir.dt.bfloat16
    fp16 = mybir.dt.float16
    i16 = mybir.dt.int16
    AF = mybir.ActivationFunctionType
    OP = mybir.AluOpType
    with (
        tc.tile_pool(name="chain", bufs=1) as chain,
        tc.tile_pool(name="psum", bufs=1, space="PSUM") as psp,
    ):
        # ---------- static loads (conv inputs first: critical path) ----------
        xc = chain.tile([128, 2, 34, 66], bf16)
        nc.scalar.dma_start(xc[:], x_conv_d[:].rearrange("c p a b -> p c a b"))
        w_om = chain.tile([128, 9, 2, 27], bf16)
        nc.scalar.dma_start(w_om[:], w_om_d[:].rearrange("k c p o -> p k c o"))
        base_y = chain.tile([128, 9, 16], f32, tag="base_y")
        nc.sync.dma_start(base_y[:], base_y_d[:])
        base_x = chain.tile([128, 9, 16], f32, tag="base_x")
        nc.sync.dma_start(base_x[:], base_x_d[:])
        bias_y = chain.tile([128, 9, 16], f32, tag="bias_y")
        nc.sync.dma_start(bias_y[:], bias_y_d[:])
        bias_x = chain.tile([128, 9, 16], f32, tag="bias_x")
        nc.sync.dma_start(bias_x[:], bias_x_d[:])
        bias_m = chain.tile([128, 9, 16], f32, tag="bias_m")
        nc.sync.dma_start(bias_m[:], bias_m_d[:])
        bn_in = chain.tile([128, 4, 2], f32, tag="bn_in")
        nc.sync.dma_start(bn_in[:], bn_d[:].rearrange("a p b -> p a b"))

        # ---------- BN constants on device ----------
        tvar = chain.tile([128, 2], f32, tag="tvar")
        nc.vector.tensor_scalar(tvar[:], bn_in[:, 3], BN_EPS, None, OP.add)
        nc.scalar.sqrt(tvar[:], tvar[:])
        nc.vector.reciprocal(tvar[:], tvar[:])
        nc.vector.tensor_tensor(bn_s[:], bn_in[:, 0], tvar[:], OP.mult)
        nc.vector.tensor_tensor(bn_o[:], bn_in[:, 2], bn_s[:], OP.mult)
        nc.vector.tensor_tensor(bn_o[:], bn_in[:, 1], bn_o[:], OP.subtract)

        # ---------- 1. offset conv ----------
        om_ps = psp.tile([27, P], f32, tag="pa")
        for ky in range(3):
            for kx in range(3):
                k = ky * 3 + kx
                for ct in range(2):
                    for n in range(4):
                        nc.tensor.matmul(
                            om_ps[:, n * 512:(n + 1) * 512],
                            w_om[:, k, ct],
                            xc[:, ct, ky + n * 8: ky + n * 8 + 8, kx: kx + 64],
                            start=(k == 0 and ct == 0),
                            stop=(k == 8 and ct == 1),
                        )
        om_sb = chain.tile([27, P], bf16, tag="om_sb")
        nc.scalar.copy(om_sb[:], om_ps[:])

        # ---------- 2. PE transpose om -> [128, 16, 27] ----------
        omT_ps = psp.tile([128, 16 * 28], bf16, tag="pb")
        for ch in range(16):
            nc.tensor.transpose(
                omT_ps[:, ch * 28:ch * 28 + 27],
                om_sb[:, ch * 128:(ch + 1) * 128],
                ident[:27, :27],
            )
        omT = chain.tile([128, 16, 27], f32, tag="omT")
        nc.scalar.copy(
            omT[:],
            omT_ps[:].rearrange("p (a b) -> p a b", a=16)[:, :, 0:27],
        )

        # ---------- 3. elementwise chain [128, 9, 16] ----------
        def ct_(name):
            return chain.tile([128, 9, 16], f32, tag=name, name=name)

        dy = ct_("dy"); dx = ct_("dx"); mm = ct_("mm")
        omT_r = omT[:].rearrange("p c o -> p o c")
        nc.vector.tensor_copy(dy[:], omT_r[:, 0:18:2, :])
        nc.vector.tensor_copy(dx[:], omT_r[:, 1:18:2, :])
        nc.vector.tensor_copy(mm[:], omT_r[:, 18:27, :])

        t0 = ct_("t0"); t1 = ct_("t1")
        nc.vector.tensor_tensor(dy[:], dy[:], bias_y[:], OP.add)
        nc.vector.tensor_tensor(dx[:], dx[:], bias_x[:], OP.add)
        nc.vector.tensor_tensor(mm[:], mm[:], bias_m[:], OP.add)
        nc.vector.tensor_scalar(t0[:], dy[:], MAX_OFF, -MAX_OFF, OP.min, OP.max)
        nc.vector.tensor_scalar(t1[:], dx[:], MAX_OFF, -MAX_OFF, OP.min, OP.max)
        pys = ct_("pys"); pxs = ct_("pxs")
        nc.vector.tensor_tensor(pys[:], t0[:], base_y[:], OP.add)
        nc.vector.tensor_tensor(pxs[:], t1[:], base_x[:], OP.add)
        ly = ct_("ly"); lx = ct_("lx")
        y0 = ct_("y0"); x0 = ct_("x0")
        iy = chain.tile([128, 9, 16], mybir.dt.int32, tag="iy", name="iy")
        ix = chain.tile([128, 9, 16], mybir.dt.int32, tag="ix", name="ix")
        # floor(pys) robust to converter rounding mode
        nc.vector.tensor_copy(iy[:], pys[:])
        nc.vector.tensor_copy(y0[:], iy[:])
        nc.vector.tensor_tensor(t0[:], y0[:], pys[:], OP.is_gt)
        nc.vector.tensor_tensor(y0[:], y0[:], t0[:], OP.subtract)
        nc.vector.tensor_tensor(ly[:], pys[:], y0[:], OP.subtract)
        nc.vector.tensor_copy(ix[:], pxs[:])
        nc.vector.tensor_copy(x0[:], ix[:])
        nc.vector.tensor_tensor(t1[:], x0[:], pxs[:], OP.is_gt)
        nc.vector.tensor_tensor(x0[:], x0[:], t1[:], OP.subtract)
        nc.vector.tensor_tensor(lx[:], pxs[:], x0[:], OP.subtract)
        # indices first: the gather critical path starts here
        idxf = ct_("idxf")
        nc.vector.tensor_scalar(t0[:], y0[:], 128.0, None, OP.mult)
        nc.vector.tensor_tensor(idxf[:], t0[:], x0[:], OP.add)
        idx_all = chain.tile([128, 2, 9, 16], i16, tag="idx_all")
        for cr, off in enumerate([0.0, 128.0]):
            nc.vector.tensor_scalar(t1[:], idxf[:], off, None, OP.add)
            nc.vector.tensor_copy(idx_all[:, cr], t1[:])
        msk = ct_("msk")
        nc.scalar.activation(msk[:], mm[:], AF.Sigmoid)
        olx = ct_("olx"); oly32 = ct_("oly32"); ly32 = ct_("ly32")
        nc.vector.tensor_scalar(olx[:], lx[:], -1.0, 1.0, OP.mult, OP.add)
        nc.vector.tensor_scalar(oly32[:], ly[:], -1.0 / QSCALE, 1.0 / QSCALE,
                                OP.mult, OP.add)
        nc.vector.tensor_scalar(ly32[:], ly[:], 1.0 / QSCALE, None, OP.mult)
        wyt = ct_("wyt"); wyb = ct_("wyb")
        nc.vector.tensor_tensor(wyt[:], oly32[:], msk[:], OP.mult)
        nc.vector.tensor_tensor(wyb[:], ly32[:], msk[:], OP.mult)
        wf = chain.tile([128, 2, 9, 16], fp16, tag="wf")
        nc.vector.tensor_tensor(wf[:, 0], wyt[:], olx[:], OP.mult)
        nc.vector.tensor_tensor(wf[:, 1], wyb[:], olx[:], OP.mult)
        # hi-byte weights carry the extra /256 (unpacked hi = 256*s8); bf16
        # keeps w/8192 in normal range (fp16 would go subnormal)
        wfh = chain.tile([128, 2, 9, 16], bf16, tag="wfh")
        t8 = ct_("t8")
        nc.vector.tensor_scalar(t8[:], wyt[:], 1.0 / 256.0, None, OP.mult)
        nc.vector.tensor_tensor(wfh[:, 0], t8[:], lx[:], OP.mult)
        nc.vector.tensor_scalar(t8[:], wyb[:], 1.0 / 256.0, None, OP.mult)
        nc.vector.tensor_tensor(wfh[:, 1], t8[:], lx[:], OP.mult)
        # bias-correction tile: sigma * (1-lx), fp16
        mcorr = chain.tile([128, 9, 16], fp16, tag="mcorr")
        nc.vector.tensor_tensor(mcorr[:], msk[:], olx[:], OP.mult)

        # ---------- 4. wrap16 indices (2 streams: top pair, bottom pair) ----
        # wrap16[q, k, cr*128 + ph*16 + fl] = idx_all[ph*16+q, cr, k, fl]
        wrap16 = chain.tile([16, 9, 2, 8, 16], i16, tag="wrap16")
        for ph in range(8):
            for cr in range(2):
                eng = nc.sync if (ph * 2 + cr) % 2 == 0 else nc.scalar
                eng.dma_start(
                    wrap16[:, :, cr, ph, :],
                    idx_all[ph * 16:(ph + 1) * 16, cr],
                )
        for g8 in range(8):
            eng = nc.sync if g8 % 2 == 0 else nc.scalar
            eng.dma_start(
                wrap_rep[g8 * 16:(g8 + 1) * 16],
                wrap16[:].rearrange("q k cr ph fl -> q k (cr ph fl)"),
            )

        # ---------- 5. weight transpose + staging; mT transpose ----------
        for k in range(9):
            wT_ps = psp.tile([16, 2 * 128], fp16, tag="pc", name=f"wT_ps{k}")
            wTh_ps = psp.tile([16, 2 * 128], bf16, tag="pb", name=f"wTh_ps{k}")
            for cr in range(2):
                nc.tensor.transpose(
                    wT_ps[:, cr * 128:(cr + 1) * 128],
                    wf[:, cr, k, :],
                    identh[:],
                )
                nc.tensor.transpose(
                    wTh_ps[:, cr * 128:(cr + 1) * 128],
                    wfh[:, cr, k, :],
                    ident[:],
                )
            wT = chain.tile([16, 2, 8, 16], fp16, tag="wT", name=f"wT{k}")
            nc.scalar.copy(wT[:].rearrange("p a b c -> p (a b c)"), wT_ps[:])
            wTh = chain.tile([16, 2, 8, 16], bf16, tag="wTh", name=f"wTh{k}")
            nc.scalar.copy(wTh[:].rearrange("p a b c -> p (a b c)"), wTh_ps[:])
            eng = nc.sync if k % 2 == 0 else nc.scalar
            eng.dma_start(
                w_stage_d[k].rearrange("(cr ph fl pl) -> fl cr ph pl", cr=2, ph=8, fl=16),
                wT[:],
            )
            eng2 = nc.scalar if k % 2 == 0 else nc.sync
            eng2.dma_start(
                w_stageh_d[k].rearrange("(cr ph fl pl) -> fl cr ph pl", cr=2, ph=8, fl=16),
                wTh[:],
            )
        # mT[j, f, p] = mcorr[p, j, f]  (pix = f*128 + p)
        mT_ps = psp.tile([9, 16 * 128], fp16, tag="pd")
        for f in range(16):
            nc.tensor.transpose(
                mT_ps[:, f * 128:(f + 1) * 128],
                mcorr[:, :, f],
                identh[:],
            )
        nc.scalar.copy(mT[:].rearrange("p a b -> p (a b)"), mT_ps[:])


def _phase2(nc, tc, mybir, x_tok, wd, ident, wrap_rep, mT, s4sb, cst,
            bn_s, bn_o, big2_outer, w_stage_d, w_stageh_d, out_d):
    f32 = mybir.dt.float32
    bf16 = mybir.dt.bfloat16
    fp16 = mybir.dt.float16
    i16 = mybir.dt.int16
    AF = mybir.ActivationFunctionType
    OP = mybir.AluOpType
    with (
        tc.tile_pool(name="big2", bufs=1) as big2,
        tc.tile_pool(name="gbuf", bufs=2) as gbuf,
        tc.tile_pool(name="wrepp", bufs=2) as wrepp,
        tc.tile_pool(name="colp", bufs=2) as colp,
        tc.tile_pool(name="tmp", bufs=2) as tmpp,
        tc.tile_pool(name="psum2", bufs=1, space="PSUM") as psp2,
    ):
        out_ps = [psp2.tile([128, P], f32, tag=f"o{ot}", name=f"out_ps{ot}")
                  for ot in range(2)]
        # bias-correction matmuls open the PSUM accumulation groups
        # chain pixel (p=ph*16+q, fl) sits at einsum free pos ph*256+fl*16+q
        mT_r = mT[:].rearrange("p fl (ph q) -> p ph fl q", ph=8)
        for ot in range(2):
            for n in range(4):
                nc.tensor.matmul(
                    out_ps[ot][:, n * 512:(n + 1) * 512],
                    s4sb[:, ot],
                    mT_r[:, 2 * n:2 * n + 2],
                    start=True,
                    stop=False,
                )
        for k in range(9):
            w_rep = wrepp.tile([128, 2 * P], fp16, tag="w_rep", name=f"w_rep{k}")
            nc.sync.dma_start(
                w_rep[:],
                w_stage_d[k].partition_broadcast(128),
            )
            w_reph = wrepp.tile([128, 2 * P], bf16, tag="w_reph", name=f"w_reph{k}")
            nc.sync.dma_start(
                w_reph[:],
                w_stageh_d[k].partition_broadcast(128),
            )
            ghs = []
            for half in range(2):
                gh = gbuf.tile([128, 2, P], i16, tag=f"g{half}", name=f"g{k}_{half}")
                nc.gpsimd.dma_gather(
                    gh[:],
                    x_tok[:].rearrange("p r c -> p (r c)"),
                    wrap_rep[:, k, half * 128:(half + 1) * 128],
                    P,
                    P,
                    256,
                    transpose=True,
                    sbuf_tokens_per_rank=128,
                    sbuf_free_dim_per_rank=512,
                    single_packet=False,
                )
                ghs.append(gh)
            col = colp.tile([128, 2, P], fp16, tag="col", name=f"col{k}")
            hi_t = tmpp.tile([128, 2, P], i16, tag="hi_t", name=f"hi_t{k}")
            hi_b = tmpp.tile([128, 2, P], i16, tag="hi_b", name=f"hi_b{k}")
            ta = tmpp.tile([128, P], fp16, tag="ta", name=f"ta{k}")
            tb = tmpp.tile([128, P], fp16, tag="tb", name=f"tb{k}")
            c255 = cst[:, 0:1]
            c8 = cst[:, 1:2]
            # unpack: hi = tok >> 8 (signed), then lo = tok & 255 in-place
            nc.vector.tensor_scalar(hi_t[:], ghs[0][:], c8, None,
                                    OP.bitwise_and)
            nc.vector.tensor_scalar(ghs[0][:], ghs[0][:], c255, None,
                                    OP.bitwise_and)
            nc.vector.tensor_scalar(hi_b[:], ghs[1][:], c8, None,
                                    OP.bitwise_and)
            nc.vector.tensor_scalar(ghs[1][:], ghs[1][:], c255, None,
                                    OP.bitwise_and)
            w00 = w_rep[:, 0:P]
            w10 = w_rep[:, P:2 * P]
            w01 = w_reph[:, 0:P]
            w11 = w_reph[:, P:2 * P]
            for ctile in range(2):
                cc = col[:, ctile]
                nc.vector.tensor_tensor(ta[:], ghs[0][:, ctile], w00, OP.mult)
                nc.vector.tensor_tensor(tb[:], hi_t[:, ctile], w01, OP.mult)
                nc.vector.tensor_tensor(cc, ta[:], tb[:], OP.add)
                nc.vector.tensor_tensor(ta[:], ghs[1][:, ctile], w10, OP.mult)
                nc.vector.tensor_tensor(tb[:], hi_b[:, ctile], w11, OP.mult)
                nc.vector.tensor_tensor(ta[:], ta[:], tb[:], OP.add)
                nc.vector.tensor_tensor(cc, cc, ta[:], OP.add)
            _emit_einsum(nc, col, wd, out_ps, k)

        # ---------- 7. BN + SiLU + unpermute + store ----------
        for ot in range(2):
            yv = big2.tile([128, P], f32, tag="yv", name=f"yv{ot}")
            sg = big2.tile([128, P], f32, tag="sg", name=f"sg{ot}")
            o_sb = big2.tile([128, P], f32, tag=f"osb{ot}", name=f"o_sb{ot}")
            nc.vector.tensor_scalar(
                yv[:], out_ps[ot][:],
                bn_s[:, ot:ot + 1], bn_o[:, ot:ot + 1],
                OP.mult, OP.add,
            )
            nc.scalar.activation(sg[:], yv[:], AF.Sigmoid)
            nc.vector.tensor_tensor(
                o_sb[:].rearrange("p (c b a) -> p c b a", c=16, b=8),
                yv[:].rearrange("p (b c a) -> p c b a", b=8, c=16),
                sg[:].rearrange("p (b c a) -> p c b a", b=8, c=16),
                OP.mult,
            )
            nc.sync.dma_start(out_d[ot], o_sb[:])


def _emit_einsum(nc, col, wd, out_ps, k):
    for ctile in range(2):
        for ot in range(2):
            for n in range(4):
                nc.tensor.matmul(
                    out_ps[ot][:, n * 512:(n + 1) * 512],
                    wd[:, k, ctile, ot],
                    col[:, ctile, n * 512:(n + 1) * 512],
                    start=False,
                    stop=(k == 8 and ctile == 1),
                )


def _prep_core_inputs(inputs, b, r):
    x = np.asarray(inputs["x"])
    w_om = np.asarray(inputs["w_om"])
    b_om = np.asarray(inputs["b_om"])
    w_dcn = np.asarray(inputs["w_dcn"])
    h0 = HL * r

    # ---- packed token grid: low byte = u8(+128) of (y,x), high = s8 of (y,x+1)
    xq_lo = np.full((GR, GC, 256), 128, dtype=np.uint16)
    xq_hi = np.zeros((GR, GC, 256), dtype=np.uint16)
    y_lo, y_hi = max(0, h0 - PAD), min(H, h0 + HL + PAD)
    xs = x[b][:, y_lo:y_hi, :].transpose(1, 2, 0)          # [rows, 64, 256]
    q = np.clip(np.rint(xs * QSCALE), -127, 127).astype(np.int16)
    r0 = y_lo - (h0 - PAD)
    r1 = y_hi - (h0 - PAD)
    xq_lo[r0:r1, PAD:PAD + W, :] = (q + 128).astype(np.uint16)
    xq_hi[r0:r1, PAD - 1:PAD + W - 1, :] = (q.astype(np.uint8).astype(np.uint16) << 8)
    xq = (xq_lo | xq_hi).view(np.int16)
    x_tok = np.ascontiguousarray(xq.swapaxes(0, 1))        # [128, 48, 256] i16

    xcv = np.zeros((256, 34, 66), dtype=BF16)
    r_lo, r_hi = max(0, h0 - 1), min(H, h0 + 33)
    xcv[:, r_lo - (h0 - 1):r_hi - (h0 - 1), 1:65] = x[b][:, r_lo:r_hi, :].astype(BF16)
    x_conv = np.ascontiguousarray(xcv.reshape(2, 128, 34, 66))

    wl = np.zeros((9, 2, 128, 27), dtype=BF16)
    for ky in range(3):
        for kx in range(3):
            k = ky * 3 + kx
            for ctile in range(2):
                wl[k, ctile] = w_om[:, ctile * 128:(ctile + 1) * 128, ky, kx].T.astype(BF16)

    wdl = np.zeros((9, 2, 2, 128, 128), dtype=FP16)
    wr = w_dcn.reshape(C2, C1, 9)
    for k in range(9):
        for ctile in range(2):
            for ot in range(2):
                wdl[k, ctile, ot] = wr[ot * 128:(ot + 1) * 128,
                                       ctile * 128:(ctile + 1) * 128, k].T.astype(FP16)

    # correction lhsT: s4[ot, k, o] = -4 * sum_c W[o, c, k]
    S = w_dcn.reshape(C2, C1, 9).sum(axis=1)               # [C2, 9]
    s4 = np.zeros((2, 9, 128), dtype=FP16)
    for ot in range(2):
        s4[ot] = (-4.0 * S[ot * 128:(ot + 1) * 128, :].T).astype(FP16)

    p_ = np.arange(128)[:, None, None]
    k_ = np.arange(9)[None, :, None]
    fl = np.arange(16)[None, None, :]
    pix = fl * 128 + p_
    h_loc = pix // W
    w_pix = pix % W
    ky_ = k_ // 3
    kx_ = k_ % 3
    base_y = np.broadcast_to(h_loc + ky_ - 1 + PAD, (128, 9, 16)).astype(np.float32)
    base_x = np.broadcast_to(w_pix + kx_ - 1 + PAD, (128, 9, 16)).astype(np.float32)
    bias_y = np.broadcast_to(b_om[0:18:2][None, :, None], (128, 9, 16)).astype(np.float32)
    bias_x = np.broadcast_to(b_om[1:18:2][None, :, None], (128, 9, 16)).astype(np.float32)
    bias_m = np.broadcast_to(b_om[18:27][None, :, None], (128, 9, 16)).astype(np.float32)

    bn = np.stack([
        np.asarray(inputs["bn_gamma"]).reshape(2, 128).T,
        np.asarray(inputs["bn_beta"]).reshape(2, 128).T,
        np.asarray(inputs["bn_mean"]).reshape(2, 128).T,
        np.asarray(inputs["bn_var"]).reshape(2, 128).T,
    ], axis=0).astype(np.float32)

    cstv = np.tile(np.array([[255, -256]], np.int16), (128, 1))

    return {
        "x_tok": x_tok,
        "x_conv": x_conv,
        "w_om": wl,
        "w_dcn": wdl,
        "s4": s4,
        "base_y": np.ascontiguousarray(base_y),
        "base_x": np.ascontiguousarray(base_x),
        "bias_y": np.ascontiguousarray(bias_y),
        "bias_x": np.ascontiguousarray(bias_x),
        "bias_m": np.ascontiguousarray(bias_m),
        "ident": np.eye(128, dtype=BF16),
        "identh": np.eye(128, dtype=FP16),
        "cst": cstv,
        "bn": np.ascontiguousarray(bn),
    }


_NC_CACHE = {}


def _get_nc():
    if "nc" not in _NC_CACHE:
        _NC_CACHE["nc"] = _build_nc()
    return _NC_CACHE["nc"]


def _assemble(results):
    out = np.zeros((B, C2, H, W), dtype=np.float32)
    for c in range(NCORES):
        b, r = c // 2, c % 2
        o = np.asarray(results[c]["out"])
        for ot in range(2):
            out[b, ot * 128:(ot + 1) * 128, HL * r:HL * (r + 1), :] = (
                o[ot].reshape(128, HL, W).astype(np.float32)
            )
    return out


def _run(inputs, trace=False):
    from concourse.bass_utils import run_bass_kernel_spmd
    nc = _get_nc()
    in_maps = [_prep_core_inputs(inputs, c // 2, c % 2) for c in range(NCORES)]
    res = run_bass_kernel_spmd(nc, in_maps, list(range(NCORES)), trace=trace)
    return _assemble(res.results), res


def kernel(**inputs):
    out, _ = _run(inputs, trace=False)
    return out
